# revision 40
# baseline (speedup 1.0000x reference)
"""Trainium2 Bass kernel for MockFP8Linear: out = x @ (W * block_scale)^T.

Strategy: data-parallel over tokens across 8 NeuronCores (no collectives).

Layout: the PE contracts along the partition dim, so both operands need
in_features on partitions. Both are fed to the device pre-transposed as
host-side layout prep (np.ascontiguousarray(.T) + bf16 cast, exactly the
prep class the baseline already used for W):
  - weight: [in, out] bf16. Dequant (per-128x128-block scale) happens
    on-device in one DVE tensor_tensor multiply per k-row, using a
    stride-0 broadcast AP for the scales. W^T (bf16, 8 MB) stays
    resident in SBUF.
  - x: tile-blocked transposed bf16 per-core shard, xb[t, p, kb, m] =
    x[t*128+m, kb*128+p], so each token tile is ONE [128, 4KB-run] DMA
    (DMA engines are packet-rate bound: 256B-run block DMAs measured
    ~6 GB/s/engine vs ~24 GB/s at 4KB runs) and lhsT blocks slice
    straight out of SBUF. No on-device transpose or cast: the
    TensorEngine runs a pure matmul stream.

Main compute runs as TWO PASSES over output halves so the prologue only
gates on half the W dequant: pass A computes out[:, 0:1024] for all 16
token tiles (the first four tiles interleaved k-block-by-k-block so the
PE chases the W-half-row DMA arrivals), pass B computes out[:, 1024:]
as a pure matmul stream over the fully resident operands. Per (tile,
k-block): lhsT(=x^T block, stationary) @ rhs(=W^T slice, moving, N=512)
bf16 matmuls accumulate fp32 into 2 PSUM banks per tile-half (4 tile
groups in flight).

Engine/queue discipline (each lesson trace-measured):
  - The critical prologue payload (x0/x1 split in half-strips + the 16
    pass-A W half-rows, in strict consumption order) rides the sync and
    scalar DMA queues only: per-queue throughput is ~100-180 GB/s
    regardless of engine idle time, and the gpsimd queue delivers its
    first bytes ~10us later than the other two.
  - Pass-A dequant runs on DVE (1.2us per half-row; GPSIMD tensor_tensor
    measured 2.5x-slowing CONCURRENT DVE ops, so it gets none).
  - start=True matmuls wait on coarsened per-engine op-COUNT semaphores,
    so DVE carries evictions + prologue dequants ONLY, in consumption
    order; anything slower-paced on that engine falsely gates PSUM
    bank reuse.
  - Pass-B dequant therefore runs on the otherwise-idle ACT engine as
    per-128-block muls with a [P,1] per-partition scale AP (ACT's scale
    operand can't vary along free dims), woven 2 rows per pass-A close.
  - Pass-B W quads + the x-tile bulk are woven across pass A on the
    gpsimd/sync queues, paced to land just ahead of their consumers;
    all out-DMAs ride sync (demand-paced) to avoid head-of-line
    blocking behind the quads.
  - A short burst of fp32 warmup matmuls off a memset scratch tile
    (no DMA dependency, so it runs ~7-11us and finishes before the first
    real matmul's operands land) pre-warms the HAM clock gate (PE
    defaults to 1.2 GHz until ~3.4us of sustained activity); the last
    tile-half is chunk-pipelined with its drain split across queues to
    shrink the tail. W rows 12-15 (consumed last) ride the slow-starting
    gpsimd queue, freeing 1MB of cold sync/scalar queue time.
"""

import os
import sys

import numpy as np

for _p in ("/opt/trn_rl_repo", "/root/.axon_site/_ro/trn_rl_repo"):
    if os.path.isdir(_p) and _p not in sys.path:
        sys.path.append(_p)

TOKENS, IN_F, OUT_F = 16384, 2048, 2048
NCORES = 8
TSH = TOKENS // NCORES  # tokens per core
P = 128
KB = IN_F // P  # contraction blocks
TB = TSH // P  # token tiles per core
OBL = OUT_F // P  # out_features blocks (scale granularity)
NCH = OUT_F // 512  # psum chunks of the output row-tile

_cached = None


def _build():
    from contextlib import ExitStack

    import concourse.tile as tile
    from concourse import bacc, mybir
    from concourse.bass import ds

    f32 = mybir.dt.float32
    bf16 = mybir.dt.bfloat16

    nc = bacc.Bacc("TRN2", target_bir_lowering=False, debug=False, num_devices=NCORES)
    xb_d = nc.dram_tensor("xb", [TB * P, IN_F], bf16, kind="ExternalInput").ap()
    wt_d = nc.dram_tensor("wt", [IN_F, OUT_F], bf16, kind="ExternalInput").ap()
    s_d = nc.dram_tensor("s", [P, KB, OBL], f32, kind="ExternalInput").ap()
    o_d = nc.dram_tensor("out", [TSH, OUT_F], f32, kind="ExternalOutput").ap()

    with tile.TileContext(nc) as tc:
        with ExitStack() as ctx:
            const = ctx.enter_context(tc.tile_pool(name="const", bufs=1))
            scales = const.tile([P, KB, OBL], f32)
            nc.scalar.dma_start(scales[:], s_d[:])
            # warmup scratch: memset needs no DMA, so the HAM warmup matmuls
            # can start at ~7us (right after the DVE preamble) and finish
            # BEFORE the first real matmul's operands land (~11.3us) —
            # sourcing them from the scales tile measured the warmup
            # BLOCKING ~3.8us of ready real work
            wscr = const.tile([P, 512], f32)
            nc.vector.memset(wscr[:], 0.0)

            wT_pool = ctx.enter_context(tc.tile_pool(name="wT", bufs=1))
            wTs = [wT_pool.tile([P, OUT_F], bf16, name=f"wT_{ib}") for ib in range(KB)]

            wnat_pool = ctx.enter_context(tc.tile_pool(name="wnat", bufs=6))
            x_pool = ctx.enter_context(tc.tile_pool(name="x", bufs=1))
            outsb_pool = ctx.enter_context(tc.tile_pool(name="outsb", bufs=3))
            ps_pool = ctx.enter_context(tc.tile_pool(name="ps", bufs=8, space="PSUM"))

            HW = OUT_F // 2  # output columns per pass

            def dequant(kb, src, lo, w):
                # wTs[kb][:, lo:lo+w] = src[:, 0:w] * scale; src 2D contiguous
                nb = w // P
                nc.vector.tensor_tensor(
                    out=wTs[kb][:, ds(lo, w)].rearrange("p (b c) -> p b c", c=P),
                    in0=src[:, ds(0, w)].rearrange("p (b c) -> p b c", c=P),
                    in1=scales[:, kb, ds(lo // P, nb), None].broadcast_to([P, nb, P]),
                    op=mybir.AluOpType.mult,
                )

            def emit_wa_group(kb0, nrows, trig, chunks=1):
                # one trigger (~0.7us of engine time) loads nrows rows' pass-A
                # halves; each row stays a 2D contiguous [P, HW] slice
                wg = wnat_pool.tile(
                    [P, nrows * HW], bf16, tag="wnat", name=f"wa_{kb0}"
                )
                if nrows == 1:
                    cw = HW // chunks
                    for j in range(chunks):
                        trig.dma_start(
                            wg[:, ds(j * cw, cw)],
                            wt_d[ds(kb0 * P, P), ds(j * cw, cw)],
                        )
                        dequant(kb0, wg[:, ds(j * cw, cw)], j * cw, cw)
                    return
                trig.dma_start(
                    wg[:].rearrange("p (r o) -> p r o", o=HW),
                    wt_d[ds(kb0 * P, nrows * P), ds(0, HW)].rearrange(
                        "(r p) o -> p r o", p=P
                    ),
                )
                for r in range(nrows):
                    dequant(kb0 + r, wg[:, ds(r * HW, HW)], 0, HW)

            wbs = {}

            def emit_wb_load(kb0, nrows):
                # pass-B halves: queued on gpsimd right AFTER the critical
                # pass-A payload (ordered queues => no early contention)
                wg = wnat_pool.tile(
                    [P, nrows * HW], bf16, tag="wnat", name=f"wb_{kb0}"
                )
                nc.gpsimd.dma_start(
                    wg[:].rearrange("p (r o) -> p r o", o=HW),
                    wt_d[ds(kb0 * P, nrows * P), ds(HW, HW)].rearrange(
                        "(r p) o -> p r o", p=P
                    ),
                )
                for r in range(nrows):
                    wbs[kb0 + r] = wg[:, ds(r * HW, HW)]

            def dequant_b_row(kb):
                # pass-B dequant on ACT as per-128-block muls with a [P,1]
                # per-partition scale AP (~0.5us each, ACT is otherwise
                # idle, and DVE must stay evictions-only)
                src_row = wbs[kb]
                for bo in range(OBL // 2):
                    nc.scalar.mul(
                        wTs[kb][:, ds(HW + bo * P, P)],
                        src_row[:, ds(bo * P, P)],
                        scales[:, kb, HW // P + bo, None],
                    )

            xtiles = {}

            def emit_x_tile(t, trig=None):
                xt = x_pool.tile([P, IN_F], bf16, name=f"x_{t}")
                (trig or nc.sync).dma_start(xt[:], xb_d[ds(t * P, P), :])
                xtiles[t] = xt

            psums = {}

            def open_group(t):
                psums[t] = [
                    ps_pool.tile([P, 512], f32, tag="ps", name=f"ps_{t}_{c}")
                    for c in range(2)
                ]

            def mm_one(t, kb, half, c):
                nc.tensor.matmul(
                    psums[t][c][:],
                    lhsT=lhsT_of(t, kb),
                    rhs=wTs[kb][:, ds(half * HW + c * 512, 512)],
                    start=(kb == 0),
                    stop=(kb == KB - 1),
                )

            def mm(t, kb, half):
                mm_one(t, kb, half, 0)
                mm_one(t, kb, half, 1)

            def close_tile(t, half):
                # both evictions on DVE: start=True matmuls wait on coarsened
                # DVE op-COUNT semaphores, so the eviction engine's stream
                # must contain nothing slower-paced than evictions
                outsb = outsb_pool.tile([P, HW], f32, tag="osb", name=f"osb_{t}_{half}")
                nc.vector.tensor_copy(outsb[:, ds(0, 512)], psums[t][0][:])
                nc.vector.tensor_copy(outsb[:, ds(512, 512)], psums[t][1][:])
                # all outs ride the sync queue: demand-paced (~0.5MB per
                # 6.8us), and keeping them off gpsimd avoids head-of-line
                # blocking behind the pass-B W quads
                nc.sync.dma_start(o_d[ds(t * P, P), ds(half * HW, HW)], outsb[:])
                del psums[t]

            # ---- prologue: the critical payload (x0-x5 + pass-A W halves,
            # ~7MB) is spread across all three DMA queues — each queue tops
            # out at ~180 GB/s, so one queue alone would gate the prologue
            # at ~30us. Row 0 is chunked for the earliest first matmul;
            # grouped W DMAs keep trigger serialization (~0.7us each) off
            # the pacing path.
            # Each queue moves only ~100 GB/s when all three are active, so
            # the payload is striped round-robin across sync/scalar/gpsimd
            # in strict consumption order (the DVE wait conditions are
            # coarsened to op-counts, so out-of-order arrival cascades).
            # x0/x1 are split into k-strips so their first k-blocks land in
            # ~1us instead of ~5us.
            # the gpsimd queue delivers its first bytes ~10us later than
            # sync/scalar, so the critical payload rides those two only
            xsplit = {}

            def make_x_split(t):
                # separate lo/hi tiles: matmul deps on an x tile are
                # coarsened to ALL its strip DMAs, so k-blocks 0-7 must not
                # wait for the later-arriving kb8-15 half
                lo = x_pool.tile([P, IN_F // 2], bf16, name=f"x_{t}lo")
                hi = x_pool.tile([P, IN_F // 2], bf16, name=f"x_{t}hi")
                xsplit[t] = (lo, hi)
                return lo, hi

            def lhsT_of(t, kb):
                if t in xsplit:
                    lo, hi = xsplit[t]
                    if kb < KB // 2:
                        return lo[:, ds(kb * P, P)]
                    return hi[:, ds((kb - KB // 2) * P, P)]
                return xtiles[t][:, ds(kb * P, P)]

            x0lo, x0hi = make_x_split(0)
            x1lo, x1hi = make_x_split(1)
            nc.sync.dma_start(x0lo[:], xb_d[ds(0, P), ds(0, IN_F // 2)])
            emit_wa_group(0, 1, nc.sync)
            # both hi strips lead the slow-starting gpsimd queue (needed
            # only at ~26us); keeping them off sync/scalar advances every
            # W row by 0.25MB of cold queue time
            nc.gpsimd.dma_start(x1hi[:], xb_d[ds(P, P), ds(IN_F // 2, IN_F // 2)])
            nc.gpsimd.dma_start(x0hi[:], xb_d[ds(0, P), ds(IN_F // 2, IN_F // 2)])
            nc.sync.dma_start(x1lo[:], xb_d[ds(P, P), ds(0, IN_F // 2)])
            emit_wa_group(1, 1, nc.scalar)
            for kb in range(2, 12):
                emit_wa_group(kb, 1, nc.sync if kb % 2 == 0 else nc.scalar)
            # rows 12-15 are consumed last (~33-39us): they follow on gpsimd
            for kb in range(12, KB):
                emit_wa_group(kb, 1, nc.gpsimd)
            emit_x_tile(2)                      # sync
            emit_x_tile(3, nc.scalar)
            emit_x_tile(4, nc.gpsimd)
            emit_x_tile(5)                      # sync

            # PE/HAM warmup: a few fp32 matmuls off the scales tile (first
            # data to arrive) into a scratch PSUM bank, so the HAM clock
            # gate is already at 8/8 when the real stream starts
            warm = ps_pool.tile([P, 512], f32, tag="ps", name="warm")
            for _ in range(5):
                nc.tensor.matmul(
                    warm[:, ds(0, 256)], lhsT=wscr[:, ds(0, P)],
                    rhs=wscr[:, ds(0, 256)], start=True, stop=True,
                )

            # ---- pass A over out[:, 0:1024]: first four tiles interleaved
            # k-block-by-k-block (8 matmuls = ~1.7us of PE work per arriving
            # W row) so the PE never starves during the W load phase.
            for t in range(4):
                open_group(t)
            for kb in range(KB):
                for t in range(4):
                    mm(t, kb, 0)

            def weave(t):
                # spread the x-tile bulk and pass-B W quads across pass A's
                # eviction stream, paced so each quad is in before its ACT
                # dequant rows come up and each x tile before its matmuls:
                # gpsimd carries x6-x9 + the quads, sync carries x10-x15
                # (ahead of that close's demand-paced out-DMA)
                if t == 0:
                    emit_x_tile(6, nc.gpsimd)
                    emit_x_tile(7, nc.gpsimd)
                    emit_wb_load(0, 4)
                elif t == 1:
                    emit_wb_load(4, 4)
                    emit_x_tile(8, nc.gpsimd)
                elif t == 2:
                    emit_wb_load(8, 4)
                    emit_x_tile(9, nc.gpsimd)
                    emit_x_tile(10)
                    emit_x_tile(11)
                elif t == 3:
                    emit_wb_load(12, 4)
                    emit_x_tile(12)
                    emit_x_tile(13)
                elif t == 4:
                    emit_x_tile(14)
                    emit_x_tile(15)
                if 2 * t + 1 < KB:
                    dequant_b_row(2 * t)
                    dequant_b_row(2 * t + 1)

            for t in range(4):
                weave(t)
                close_tile(t, 0)

            for t in range(4, TB):
                open_group(t)
                for kb in range(KB):
                    mm(t, kb, 0)
                weave(t)
                close_tile(t, 0)

            # ---- pass B over out[:, 1024:2048]: pure matmul stream ----
            for t in range(TB):
                open_group(t)
                last = t == TB - 1
                if not last:
                    for kb in range(KB):
                        mm(t, kb, 1)
                    close_tile(t, 1)
                else:
                    # chunk-outer on the final tile so the drain pipelines
                    outsb = outsb_pool.tile([P, HW], f32, tag="osb", name="osb_last")
                    for c in range(2):
                        for kb in range(KB):
                            mm_one(t, kb, 1, c)
                        if c == 0:
                            nc.vector.tensor_copy(
                                outsb[:, ds(0, 512)], psums[t][0][:]
                            )
                            nc.gpsimd.dma_start(
                                o_d[ds(t * P, P), ds(HW, 512)], outsb[:, ds(0, 512)]
                            )
                        else:
                            # final chunk: evict + drain split across queues
                            nc.vector.tensor_copy(
                                outsb[:, ds(512, 256)], psums[t][1][:, ds(0, 256)]
                            )
                            nc.scalar.copy(
                                outsb[:, ds(768, 256)], psums[t][1][:, ds(256, 256)]
                            )
                            nc.sync.dma_start(
                                o_d[ds(t * P, P), ds(HW + 512, 256)],
                                outsb[:, ds(512, 256)],
                            )
                            nc.scalar.dma_start(
                                o_d[ds(t * P, P), ds(HW + 768, 256)],
                                outsb[:, ds(768, 256)],
                            )
                    del psums[t]

    nc.compile()
    return nc


def _get_compiled():
    global _cached
    if _cached is None:
        _cached = _build()
    return _cached


def _ensure_ntff_hook():
    """Register the axon NTFF profile hook (boot skips it when
    antenv.axon_hooks is absent from the image). Only needed for trace=True."""
    import sys as _sys
    import types as _types

    if "antenv.axon_hooks" not in _sys.modules:
        import antenv

        mod = _types.ModuleType("antenv.axon_hooks")
        mod._hook = None

        def set_axon_ntff_profile_hook(h):
            mod._hook = h

        def get_axon_ntff_profile_hook():
            return mod._hook

        mod.set_axon_ntff_profile_hook = set_axon_ntff_profile_hook
        mod.get_axon_ntff_profile_hook = get_axon_ntff_profile_hook
        _sys.modules["antenv.axon_hooks"] = mod
        antenv.axon_hooks = mod
    mod = _sys.modules["antenv.axon_hooks"]
    if mod._hook is None:
        from trn_agent_boot.trn_boot import _ntff_profile_via_ctypes

        hook = _ntff_profile_via_ctypes("/opt/axon/libaxon_pjrt.so")
        if hook is not None:
            mod.set_axon_ntff_profile_hook(hook)


def run(x, weight, weight_scale, trace=False, trace_cores=None):
    from concourse.bass_utils import run_bass_kernel_spmd

    nc = _get_compiled()

    import ml_dtypes

    bf16 = ml_dtypes.bfloat16
    x = np.asarray(x, dtype=np.float32)
    weight = np.asarray(weight, dtype=np.float32)
    wt = np.ascontiguousarray(weight.T.astype(bf16))
    weight_scale = np.asarray(weight_scale, dtype=np.float32)
    # [P, KB(bi), OBL(bo)]: s[p, bi, bo] = weight_scale[bo, bi]
    scales_b = np.ascontiguousarray(
        np.broadcast_to(weight_scale.T[None, :, :], (P, KB, OBL)).astype(np.float32)
    )

    def blocked_x(shard):
        # xb[t, p, kb, m] = shard[t*128+m, kb*128+p]  (layout prep only)
        xb = shard.reshape(TB, P, KB, P).transpose(0, 3, 2, 1)
        return np.ascontiguousarray(xb.astype(bf16).reshape(TB * P, IN_F))

    in_maps = [
        {
            "xb": blocked_x(x[c * TSH : (c + 1) * TSH]),
            "wt": wt,
            "s": scales_b,
        }
        for c in range(NCORES)
    ]
    kwargs = {}
    if trace:
        try:
            _ensure_ntff_hook()
        except Exception as e:  # tracing is best-effort; the run still works
            print(f"ntff hook registration failed ({e}); tracing may be skipped")
        kwargs = dict(trace=True, trace_cores=trace_cores or [0])
    res = run_bass_kernel_spmd(nc, in_maps, core_ids=list(range(NCORES)), **kwargs)
    out = np.concatenate([res.results[c]["out"] for c in range(NCORES)], axis=0)
    return out, res


def kernel(x, weight, weight_scale):
    # Rare transient device errors (NRT_EXEC_UNIT_UNRECOVERABLE) have been
    # observed under the profiling path; retry once to be safe.
    try:
        out, _ = run(x, weight, weight_scale)
    except Exception:
        import time

        time.sleep(2)
        out, _ = run(x, weight, weight_scale)
    return out


# revision 42
# speedup vs baseline: 1.0040x; 1.0040x over previous
"""Trainium2 Bass kernel for MockFP8Linear: out = x @ (W * block_scale)^T.

Strategy: data-parallel over tokens across 8 NeuronCores (no collectives).

Layout: the PE contracts along the partition dim, so both operands need
in_features on partitions. Both are fed to the device pre-transposed as
host-side layout prep (np.ascontiguousarray(.T) + bf16 cast, exactly the
prep class the baseline already used for W):
  - weight: [in, out] bf16. Dequant (per-128x128-block scale) happens
    on-device in one DVE tensor_tensor multiply per k-row, using a
    stride-0 broadcast AP for the scales. W^T (bf16, 8 MB) stays
    resident in SBUF.
  - x: tile-blocked transposed bf16 per-core shard, xb[t, p, kb, m] =
    x[t*128+m, kb*128+p], so each token tile is ONE [128, 4KB-run] DMA
    (DMA engines are packet-rate bound: 256B-run block DMAs measured
    ~6 GB/s/engine vs ~24 GB/s at 4KB runs) and lhsT blocks slice
    straight out of SBUF. No on-device transpose or cast: the
    TensorEngine runs a pure matmul stream.

Main compute runs as TWO PASSES over output halves so the prologue only
gates on half the W dequant: pass A computes out[:, 0:1024] for all 16
token tiles (the first four tiles interleaved k-block-by-k-block so the
PE chases the W-half-row DMA arrivals), pass B computes out[:, 1024:]
as a pure matmul stream over the fully resident operands. Per (tile,
k-block): lhsT(=x^T block, stationary) @ rhs(=W^T slice, moving, N=512)
bf16 matmuls accumulate fp32 into 2 PSUM banks per tile-half (4 tile
groups in flight).

Engine/queue discipline (each lesson trace-measured):
  - The critical prologue payload (x0/x1 split in half-strips + the 16
    pass-A W half-rows, in strict consumption order) rides the sync and
    scalar DMA queues only: per-queue throughput is ~100-180 GB/s
    regardless of engine idle time, and the gpsimd queue delivers its
    first bytes ~10us later than the other two.
  - Pass-A dequant runs on DVE (1.2us per half-row; GPSIMD tensor_tensor
    measured 2.5x-slowing CONCURRENT DVE ops, so it gets none).
  - start=True matmuls wait on coarsened per-engine op-COUNT semaphores,
    so DVE carries evictions + prologue dequants ONLY, in consumption
    order; anything slower-paced on that engine falsely gates PSUM
    bank reuse.
  - Pass-B dequant therefore runs on the otherwise-idle ACT engine as
    per-128-block muls with a [P,1] per-partition scale AP (ACT's scale
    operand can't vary along free dims), woven 2 rows per pass-A close.
  - Pass-B W quads + the x-tile bulk are woven across pass A on the
    gpsimd/sync queues, paced to land just ahead of their consumers;
    all out-DMAs ride sync (demand-paced) to avoid head-of-line
    blocking behind the quads.
  - A short burst of fp32 warmup matmuls off a memset scratch tile
    (no DMA dependency, so it runs ~7-11us and finishes before the first
    real matmul's operands land) pre-warms the HAM clock gate (PE
    defaults to 1.2 GHz until ~3.4us of sustained activity); the last
    tile-half is chunk-pipelined with its drain split across queues to
    shrink the tail. W rows 12-15 (consumed last) ride the slow-starting
    gpsimd queue, freeing 1MB of cold sync/scalar queue time.
"""

import os
import sys

import numpy as np

for _p in ("/opt/trn_rl_repo", "/root/.axon_site/_ro/trn_rl_repo"):
    if os.path.isdir(_p) and _p not in sys.path:
        sys.path.append(_p)

TOKENS, IN_F, OUT_F = 16384, 2048, 2048
NCORES = 8
TSH = TOKENS // NCORES  # tokens per core
P = 128
KB = IN_F // P  # contraction blocks
TB = TSH // P  # token tiles per core
OBL = OUT_F // P  # out_features blocks (scale granularity)
NCH = OUT_F // 512  # psum chunks of the output row-tile

_cached = None


def _build():
    from contextlib import ExitStack

    import concourse.tile as tile
    from concourse import bacc, mybir
    from concourse.bass import ds

    f32 = mybir.dt.float32
    bf16 = mybir.dt.bfloat16

    nc = bacc.Bacc("TRN2", target_bir_lowering=False, debug=False, num_devices=NCORES)
    xb_d = nc.dram_tensor("xb", [TB * P, IN_F], bf16, kind="ExternalInput").ap()
    wt_d = nc.dram_tensor("wt", [IN_F, OUT_F], bf16, kind="ExternalInput").ap()
    s_d = nc.dram_tensor("s", [P, KB, OBL], f32, kind="ExternalInput").ap()
    o_d = nc.dram_tensor("out", [TSH, OUT_F], f32, kind="ExternalOutput").ap()

    with tile.TileContext(nc) as tc:
        with ExitStack() as ctx:
            const = ctx.enter_context(tc.tile_pool(name="const", bufs=1))
            scales = const.tile([P, KB, OBL], f32)
            nc.scalar.dma_start(scales[:], s_d[:])
            # warmup scratch: memset needs no DMA, so the HAM warmup matmuls
            # can start at ~7us (right after the DVE preamble) and finish
            # BEFORE the first real matmul's operands land (~11.3us) —
            # sourcing them from the scales tile measured the warmup
            # BLOCKING ~3.8us of ready real work
            wscr = const.tile([P, 512], f32)
            nc.vector.memset(wscr[:], 0.0)

            wT_pool = ctx.enter_context(tc.tile_pool(name="wT", bufs=1))
            wTs = [wT_pool.tile([P, OUT_F], bf16, name=f"wT_{ib}") for ib in range(KB)]

            wnat_pool = ctx.enter_context(tc.tile_pool(name="wnat", bufs=6))
            x_pool = ctx.enter_context(tc.tile_pool(name="x", bufs=1))
            outsb_pool = ctx.enter_context(tc.tile_pool(name="outsb", bufs=3))
            ps_pool = ctx.enter_context(tc.tile_pool(name="ps", bufs=8, space="PSUM"))

            HW = OUT_F // 2  # output columns per pass

            def dequant(kb, src, lo, w):
                # wTs[kb][:, lo:lo+w] = src[:, 0:w] * scale; src 2D contiguous
                nb = w // P
                nc.vector.tensor_tensor(
                    out=wTs[kb][:, ds(lo, w)].rearrange("p (b c) -> p b c", c=P),
                    in0=src[:, ds(0, w)].rearrange("p (b c) -> p b c", c=P),
                    in1=scales[:, kb, ds(lo // P, nb), None].broadcast_to([P, nb, P]),
                    op=mybir.AluOpType.mult,
                )

            def emit_wa_group(kb0, nrows, trig, chunks=1):
                # one trigger (~0.7us of engine time) loads nrows rows' pass-A
                # halves; each row stays a 2D contiguous [P, HW] slice
                wg = wnat_pool.tile(
                    [P, nrows * HW], bf16, tag="wnat", name=f"wa_{kb0}"
                )
                if nrows == 1:
                    cw = HW // chunks
                    for j in range(chunks):
                        trig.dma_start(
                            wg[:, ds(j * cw, cw)],
                            wt_d[ds(kb0 * P, P), ds(j * cw, cw)],
                        )
                        dequant(kb0, wg[:, ds(j * cw, cw)], j * cw, cw)
                    return
                trig.dma_start(
                    wg[:].rearrange("p (r o) -> p r o", o=HW),
                    wt_d[ds(kb0 * P, nrows * P), ds(0, HW)].rearrange(
                        "(r p) o -> p r o", p=P
                    ),
                )
                for r in range(nrows):
                    dequant(kb0 + r, wg[:, ds(r * HW, HW)], 0, HW)

            wbs = {}

            def emit_wb_load(kb0, nrows):
                # pass-B halves: queued on gpsimd right AFTER the critical
                # pass-A payload (ordered queues => no early contention)
                wg = wnat_pool.tile(
                    [P, nrows * HW], bf16, tag="wnat", name=f"wb_{kb0}"
                )
                nc.gpsimd.dma_start(
                    wg[:].rearrange("p (r o) -> p r o", o=HW),
                    wt_d[ds(kb0 * P, nrows * P), ds(HW, HW)].rearrange(
                        "(r p) o -> p r o", p=P
                    ),
                )
                for r in range(nrows):
                    wbs[kb0 + r] = wg[:, ds(r * HW, HW)]

            def dequant_b_row(kb):
                # pass-B dequant on ACT as per-128-block muls with a [P,1]
                # per-partition scale AP (~0.5us each, ACT is otherwise
                # idle, and DVE must stay evictions-only)
                src_row = wbs[kb]
                for bo in range(OBL // 2):
                    nc.scalar.mul(
                        wTs[kb][:, ds(HW + bo * P, P)],
                        src_row[:, ds(bo * P, P)],
                        scales[:, kb, HW // P + bo, None],
                    )

            xtiles = {}

            def emit_x_tile(t, trig=None):
                xt = x_pool.tile([P, IN_F], bf16, name=f"x_{t}")
                (trig or nc.sync).dma_start(xt[:], xb_d[ds(t * P, P), :])
                xtiles[t] = xt

            psums = {}

            def open_group(t):
                psums[t] = [
                    ps_pool.tile([P, 512], f32, tag="ps", name=f"ps_{t}_{c}")
                    for c in range(2)
                ]

            def mm_one(t, kb, half, c):
                nc.tensor.matmul(
                    psums[t][c][:],
                    lhsT=lhsT_of(t, kb),
                    rhs=wTs[kb][:, ds(half * HW + c * 512, 512)],
                    start=(kb == 0),
                    stop=(kb == KB - 1),
                )

            def mm(t, kb, half):
                mm_one(t, kb, half, 0)
                mm_one(t, kb, half, 1)

            def close_tile(t, half):
                # both evictions on DVE: start=True matmuls wait on coarsened
                # DVE op-COUNT semaphores, so the eviction engine's stream
                # must contain nothing slower-paced than evictions
                outsb = outsb_pool.tile([P, HW], f32, tag="osb", name=f"osb_{t}_{half}")
                nc.vector.tensor_copy(outsb[:, ds(0, 512)], psums[t][0][:])
                nc.vector.tensor_copy(outsb[:, ds(512, 512)], psums[t][1][:])
                # all outs ride the sync queue: demand-paced (~0.5MB per
                # 6.8us), and keeping them off gpsimd avoids head-of-line
                # blocking behind the pass-B W quads
                nc.sync.dma_start(o_d[ds(t * P, P), ds(half * HW, HW)], outsb[:])
                del psums[t]

            # ---- prologue: the critical payload (x0-x5 + pass-A W halves,
            # ~7MB) is spread across all three DMA queues — each queue tops
            # out at ~180 GB/s, so one queue alone would gate the prologue
            # at ~30us. Row 0 is chunked for the earliest first matmul;
            # grouped W DMAs keep trigger serialization (~0.7us each) off
            # the pacing path.
            # Each queue moves only ~100 GB/s when all three are active, so
            # the payload is striped round-robin across sync/scalar/gpsimd
            # in strict consumption order (the DVE wait conditions are
            # coarsened to op-counts, so out-of-order arrival cascades).
            # x0/x1 are split into k-strips so their first k-blocks land in
            # ~1us instead of ~5us.
            # the gpsimd queue delivers its first bytes ~10us later than
            # sync/scalar, so the critical payload rides those two only
            xsplit = {}

            def make_x_split(t):
                # x0/x1 as separate lo/hi tiles: matmul deps on a tile are
                # coarsened to ALL its strip DMAs, so k-blocks 0-7 must not
                # wait for the later-arriving kb8-15 strip
                lo = x_pool.tile([P, IN_F // 2], bf16, name=f"x_{t}lo")
                hi = x_pool.tile([P, IN_F // 2], bf16, name=f"x_{t}hi")
                xsplit[t] = (lo, hi)
                return lo, hi

            def lhsT_of(t, kb):
                if t in xsplit:
                    lo, hi = xsplit[t]
                    if kb < KB // 2:
                        return lo[:, ds(kb * P, P)]
                    return hi[:, ds((kb - KB // 2) * P, P)]
                return xtiles[t][:, ds(kb * P, P)]

            x0lo, x0hi = make_x_split(0)
            x1lo, x1hi = make_x_split(1)
            nc.sync.dma_start(x0lo[:], xb_d[ds(0, P), ds(0, IN_F // 2)])
            nc.scalar.dma_start(x0hi[:], xb_d[ds(0, P), ds(IN_F // 2, IN_F // 2)])
            emit_wa_group(0, 1, nc.sync)
            emit_wa_group(1, 1, nc.scalar)
            # x1's upper strip is the gpsimd queue's FIRST item: each queue
            # serves its first ~1MB at only ~50 GB/s (cold start), and this
            # strip isn't needed until ~25us — parking it on gpsimd keeps
            # the scalar queue's W rows 0.25MB earlier
            nc.sync.dma_start(x1lo[:], xb_d[ds(P, P), ds(0, IN_F // 2)])
            nc.gpsimd.dma_start(x1hi[:], xb_d[ds(P, P), ds(IN_F // 2, IN_F // 2)])
            for kb in range(2, 12):
                emit_wa_group(kb, 1, nc.sync if kb % 2 == 0 else nc.scalar)
            # rows 12-15 are consumed last (~33-39us): park them on gpsimd
            # behind x1's strip, freeing 1MB of cold sync/scalar queue time
            for kb in range(12, KB):
                emit_wa_group(kb, 1, nc.gpsimd)
            emit_x_tile(2)                      # sync
            emit_x_tile(3, nc.scalar)
            emit_x_tile(4, nc.gpsimd)
            emit_x_tile(5)                      # sync

            # PE/HAM warmup: a few fp32 matmuls off the scales tile (first
            # data to arrive) into a scratch PSUM bank, so the HAM clock
            # gate is already at 8/8 when the real stream starts
            warm = ps_pool.tile([P, 512], f32, tag="ps", name="warm")
            for _ in range(5):
                nc.tensor.matmul(
                    warm[:, ds(0, 256)], lhsT=wscr[:, ds(0, P)],
                    rhs=wscr[:, ds(0, 256)], start=True, stop=True,
                )

            # ---- pass A over out[:, 0:1024]: first four tiles interleaved
            # k-block-by-k-block (8 matmuls = ~1.7us of PE work per arriving
            # W row) so the PE never starves during the W load phase.
            for t in range(4):
                open_group(t)
            for kb in range(KB):
                for t in range(4):
                    mm(t, kb, 0)

            def weave(t):
                # spread the x-tile bulk and pass-B W quads across pass A's
                # eviction stream, paced so each quad is in before its ACT
                # dequant rows come up and each x tile before its matmuls:
                # gpsimd carries x6-x9 + the quads, sync carries x10-x15
                # (ahead of that close's demand-paced out-DMA)
                if t == 0:
                    emit_x_tile(6, nc.gpsimd)
                    emit_x_tile(7, nc.gpsimd)
                    emit_wb_load(0, 4)
                elif t == 1:
                    emit_wb_load(4, 4)
                    emit_x_tile(8, nc.gpsimd)
                elif t == 2:
                    emit_wb_load(8, 4)
                    emit_x_tile(9, nc.gpsimd)
                    emit_x_tile(10)
                    emit_x_tile(11)
                elif t == 3:
                    emit_wb_load(12, 4)
                    emit_x_tile(12)
                    emit_x_tile(13)
                elif t == 4:
                    emit_x_tile(14)
                    emit_x_tile(15)
                if 2 * t + 1 < KB:
                    dequant_b_row(2 * t)
                    dequant_b_row(2 * t + 1)

            for t in range(4):
                weave(t)
                close_tile(t, 0)

            for t in range(4, TB):
                open_group(t)
                for kb in range(KB):
                    mm(t, kb, 0)
                weave(t)
                close_tile(t, 0)

            # ---- pass B over out[:, 1024:2048]: pure matmul stream ----
            for t in range(TB):
                open_group(t)
                last = t == TB - 1
                if not last:
                    for kb in range(KB):
                        mm(t, kb, 1)
                    close_tile(t, 1)
                else:
                    # chunk-outer on the final tile so the drain pipelines
                    outsb = outsb_pool.tile([P, HW], f32, tag="osb", name="osb_last")
                    for c in range(2):
                        for kb in range(KB):
                            mm_one(t, kb, 1, c)
                        if c == 0:
                            nc.vector.tensor_copy(
                                outsb[:, ds(0, 512)], psums[t][0][:]
                            )
                            nc.gpsimd.dma_start(
                                o_d[ds(t * P, P), ds(HW, 512)], outsb[:, ds(0, 512)]
                            )
                        else:
                            # final chunk: evict + drain split across queues
                            nc.vector.tensor_copy(
                                outsb[:, ds(512, 256)], psums[t][1][:, ds(0, 256)]
                            )
                            nc.scalar.copy(
                                outsb[:, ds(768, 256)], psums[t][1][:, ds(256, 256)]
                            )
                            nc.sync.dma_start(
                                o_d[ds(t * P, P), ds(HW + 512, 256)],
                                outsb[:, ds(512, 256)],
                            )
                            nc.scalar.dma_start(
                                o_d[ds(t * P, P), ds(HW + 768, 256)],
                                outsb[:, ds(768, 256)],
                            )
                    del psums[t]

    nc.compile()
    return nc


def _get_compiled():
    global _cached
    if _cached is None:
        _cached = _build()
    return _cached


def _ensure_ntff_hook():
    """Register the axon NTFF profile hook (boot skips it when
    antenv.axon_hooks is absent from the image). Only needed for trace=True."""
    import sys as _sys
    import types as _types

    if "antenv.axon_hooks" not in _sys.modules:
        import antenv

        mod = _types.ModuleType("antenv.axon_hooks")
        mod._hook = None

        def set_axon_ntff_profile_hook(h):
            mod._hook = h

        def get_axon_ntff_profile_hook():
            return mod._hook

        mod.set_axon_ntff_profile_hook = set_axon_ntff_profile_hook
        mod.get_axon_ntff_profile_hook = get_axon_ntff_profile_hook
        _sys.modules["antenv.axon_hooks"] = mod
        antenv.axon_hooks = mod
    mod = _sys.modules["antenv.axon_hooks"]
    if mod._hook is None:
        from trn_agent_boot.trn_boot import _ntff_profile_via_ctypes

        hook = _ntff_profile_via_ctypes("/opt/axon/libaxon_pjrt.so")
        if hook is not None:
            mod.set_axon_ntff_profile_hook(hook)


def run(x, weight, weight_scale, trace=False, trace_cores=None):
    from concourse.bass_utils import run_bass_kernel_spmd

    nc = _get_compiled()

    import ml_dtypes

    bf16 = ml_dtypes.bfloat16
    x = np.asarray(x, dtype=np.float32)
    weight = np.asarray(weight, dtype=np.float32)
    wt = np.ascontiguousarray(weight.T.astype(bf16))
    weight_scale = np.asarray(weight_scale, dtype=np.float32)
    # [P, KB(bi), OBL(bo)]: s[p, bi, bo] = weight_scale[bo, bi]
    scales_b = np.ascontiguousarray(
        np.broadcast_to(weight_scale.T[None, :, :], (P, KB, OBL)).astype(np.float32)
    )

    def blocked_x(shard):
        # xb[t, p, kb, m] = shard[t*128+m, kb*128+p]  (layout prep only)
        xb = shard.reshape(TB, P, KB, P).transpose(0, 3, 2, 1)
        return np.ascontiguousarray(xb.astype(bf16).reshape(TB * P, IN_F))

    in_maps = [
        {
            "xb": blocked_x(x[c * TSH : (c + 1) * TSH]),
            "wt": wt,
            "s": scales_b,
        }
        for c in range(NCORES)
    ]
    kwargs = {}
    if trace:
        try:
            _ensure_ntff_hook()
        except Exception as e:  # tracing is best-effort; the run still works
            print(f"ntff hook registration failed ({e}); tracing may be skipped")
        kwargs = dict(trace=True, trace_cores=trace_cores or [0])
    res = run_bass_kernel_spmd(nc, in_maps, core_ids=list(range(NCORES)), **kwargs)
    out = np.concatenate([res.results[c]["out"] for c in range(NCORES)], axis=0)
    return out, res


def kernel(x, weight, weight_scale):
    # Rare transient device errors (NRT_EXEC_UNIT_UNRECOVERABLE) have been
    # observed under the profiling path; retry once to be safe.
    try:
        out, _ = run(x, weight, weight_scale)
    except Exception:
        import time

        time.sleep(2)
        out, _ = run(x, weight, weight_scale)
    return out


# revision 44
# speedup vs baseline: 1.0142x; 1.0101x over previous
"""Trainium2 Bass kernel for MockFP8Linear: out = x @ (W * block_scale)^T.

Strategy: data-parallel over tokens across 8 NeuronCores (no collectives).

Layout: the PE contracts along the partition dim, so both operands need
in_features on partitions. Both are fed to the device pre-transposed as
host-side layout prep (np.ascontiguousarray(.T) + bf16 cast, exactly the
prep class the baseline already used for W):
  - weight: [in, out] bf16. Dequant (per-128x128-block scale) happens
    on-device in one DVE tensor_tensor multiply per k-row, using a
    stride-0 broadcast AP for the scales. W^T (bf16, 8 MB) stays
    resident in SBUF.
  - x: tile-blocked transposed bf16 per-core shard, xb[t, p, kb, m] =
    x[t*128+m, kb*128+p], so each token tile is ONE [128, 4KB-run] DMA
    (DMA engines are packet-rate bound: 256B-run block DMAs measured
    ~6 GB/s/engine vs ~24 GB/s at 4KB runs) and lhsT blocks slice
    straight out of SBUF. No on-device transpose or cast: the
    TensorEngine runs a pure matmul stream.

Main compute runs as TWO PASSES over output halves so the prologue only
gates on half the W dequant: pass A computes out[:, 0:1024] for all 16
token tiles (the first four tiles interleaved k-block-by-k-block so the
PE chases the W-half-row DMA arrivals), pass B computes out[:, 1024:]
as a pure matmul stream over the fully resident operands. Per (tile,
k-block): lhsT(=x^T block, stationary) @ rhs(=W^T slice, moving, N=512)
bf16 matmuls accumulate fp32 into 2 PSUM banks per tile-half (4 tile
groups in flight).

Engine/queue discipline (each lesson trace-measured):
  - The critical prologue payload (x0/x1 split in half-strips + the 16
    pass-A W half-rows, in strict consumption order) rides the sync and
    scalar DMA queues only: per-queue throughput is ~100-180 GB/s
    regardless of engine idle time, and the gpsimd queue delivers its
    first bytes ~10us later than the other two.
  - Pass-A dequant runs on DVE (1.2us per half-row; GPSIMD tensor_tensor
    measured 2.5x-slowing CONCURRENT DVE ops, so it gets none).
  - start=True matmuls wait on coarsened per-engine op-COUNT semaphores,
    so DVE carries evictions + prologue dequants ONLY, in consumption
    order; anything slower-paced on that engine falsely gates PSUM
    bank reuse.
  - Pass-B dequant therefore runs on the otherwise-idle ACT engine as
    per-128-block muls with a [P,1] per-partition scale AP (ACT's scale
    operand can't vary along free dims), woven 2 rows per pass-A close.
  - Pass-B W quads + the x-tile bulk are woven across pass A on the
    gpsimd/sync queues, paced to land just ahead of their consumers;
    all out-DMAs ride sync (demand-paced) to avoid head-of-line
    blocking behind the quads.
  - A short burst of fp32 warmup matmuls off a memset scratch tile
    (no DMA dependency, so it runs ~7-11us and finishes before the first
    real matmul's operands land) pre-warms the HAM clock gate (PE
    defaults to 1.2 GHz until ~3.4us of sustained activity); the last
    tile-half is chunk-pipelined with its drain split across queues to
    shrink the tail. W rows 12-15 (consumed last) ride the slow-starting
    gpsimd queue, freeing 1MB of cold sync/scalar queue time.
"""

import os
import sys

import numpy as np

for _p in ("/opt/trn_rl_repo", "/root/.axon_site/_ro/trn_rl_repo"):
    if os.path.isdir(_p) and _p not in sys.path:
        sys.path.append(_p)

TOKENS, IN_F, OUT_F = 16384, 2048, 2048
NCORES = 8
TSH = TOKENS // NCORES  # tokens per core
P = 128
KB = IN_F // P  # contraction blocks
TB = TSH // P  # token tiles per core
OBL = OUT_F // P  # out_features blocks (scale granularity)
NCH = OUT_F // 512  # psum chunks of the output row-tile

_cached = None


def _build():
    from contextlib import ExitStack

    import concourse.tile as tile
    from concourse import bacc, mybir
    from concourse.bass import ds

    f32 = mybir.dt.float32
    bf16 = mybir.dt.bfloat16

    nc = bacc.Bacc("TRN2", target_bir_lowering=False, debug=False, num_devices=NCORES)
    xb_d = nc.dram_tensor("xb", [TB * P, IN_F], bf16, kind="ExternalInput").ap()
    wt_d = nc.dram_tensor("wt", [IN_F, OUT_F], bf16, kind="ExternalInput").ap()
    s_d = nc.dram_tensor("s", [P, KB, OBL], f32, kind="ExternalInput").ap()
    o_d = nc.dram_tensor("out", [TSH, OUT_F], f32, kind="ExternalOutput").ap()

    with tile.TileContext(nc) as tc:
        with ExitStack() as ctx:
            const = ctx.enter_context(tc.tile_pool(name="const", bufs=1))
            scales = const.tile([P, KB, OBL], f32)
            nc.scalar.dma_start(scales[:], s_d[:])
            # warmup scratch: memset needs no DMA, so the HAM warmup matmuls
            # can start at ~7us (right after the DVE preamble) and finish
            # BEFORE the first real matmul's operands land (~11.3us) —
            # sourcing them from the scales tile measured the warmup
            # BLOCKING ~3.8us of ready real work
            wscr = const.tile([P, 512], f32)
            nc.vector.memset(wscr[:], 0.0)

            wT_pool = ctx.enter_context(tc.tile_pool(name="wT", bufs=1))
            wTs = [wT_pool.tile([P, OUT_F], bf16, name=f"wT_{ib}") for ib in range(KB)]

            wnat_pool = ctx.enter_context(tc.tile_pool(name="wnat", bufs=6))
            x_pool = ctx.enter_context(tc.tile_pool(name="x", bufs=1))
            outsb_pool = ctx.enter_context(tc.tile_pool(name="outsb", bufs=3))
            ps_pool = ctx.enter_context(tc.tile_pool(name="ps", bufs=8, space="PSUM"))

            HW = OUT_F // 2  # output columns per pass

            def dequant(kb, src, lo, w):
                # wTs[kb][:, lo:lo+w] = src[:, 0:w] * scale; src 2D contiguous
                nb = w // P
                nc.vector.tensor_tensor(
                    out=wTs[kb][:, ds(lo, w)].rearrange("p (b c) -> p b c", c=P),
                    in0=src[:, ds(0, w)].rearrange("p (b c) -> p b c", c=P),
                    in1=scales[:, kb, ds(lo // P, nb), None].broadcast_to([P, nb, P]),
                    op=mybir.AluOpType.mult,
                )

            def emit_wa_group(kb0, nrows, trig, chunks=1):
                # one trigger (~0.7us of engine time) loads nrows rows' pass-A
                # halves; each row stays a 2D contiguous [P, HW] slice
                wg = wnat_pool.tile(
                    [P, nrows * HW], bf16, tag="wnat", name=f"wa_{kb0}"
                )
                if nrows == 1:
                    cw = HW // chunks
                    for j in range(chunks):
                        trig.dma_start(
                            wg[:, ds(j * cw, cw)],
                            wt_d[ds(kb0 * P, P), ds(j * cw, cw)],
                        )
                        dequant(kb0, wg[:, ds(j * cw, cw)], j * cw, cw)
                    return
                trig.dma_start(
                    wg[:].rearrange("p (r o) -> p r o", o=HW),
                    wt_d[ds(kb0 * P, nrows * P), ds(0, HW)].rearrange(
                        "(r p) o -> p r o", p=P
                    ),
                )
                for r in range(nrows):
                    dequant(kb0 + r, wg[:, ds(r * HW, HW)], 0, HW)

            wbs = {}

            def emit_wb_load(kb0, nrows):
                # pass-B halves: queued on gpsimd right AFTER the critical
                # pass-A payload (ordered queues => no early contention)
                wg = wnat_pool.tile(
                    [P, nrows * HW], bf16, tag="wnat", name=f"wb_{kb0}"
                )
                nc.gpsimd.dma_start(
                    wg[:].rearrange("p (r o) -> p r o", o=HW),
                    wt_d[ds(kb0 * P, nrows * P), ds(HW, HW)].rearrange(
                        "(r p) o -> p r o", p=P
                    ),
                )
                for r in range(nrows):
                    wbs[kb0 + r] = wg[:, ds(r * HW, HW)]

            def dequant_b_row(kb):
                # pass-B dequant on ACT as per-128-block muls with a [P,1]
                # per-partition scale AP (~0.5us each, ACT is otherwise
                # idle, and DVE must stay evictions-only)
                src_row = wbs[kb]
                for bo in range(OBL // 2):
                    nc.scalar.mul(
                        wTs[kb][:, ds(HW + bo * P, P)],
                        src_row[:, ds(bo * P, P)],
                        scales[:, kb, HW // P + bo, None],
                    )

            xtiles = {}

            def emit_x_tile(t, trig=None):
                xt = x_pool.tile([P, IN_F], bf16, name=f"x_{t}")
                (trig or nc.sync).dma_start(xt[:], xb_d[ds(t * P, P), :])
                xtiles[t] = xt

            psums = {}

            def open_group(t):
                psums[t] = [
                    ps_pool.tile([P, 512], f32, tag="ps", name=f"ps_{t}_{c}")
                    for c in range(2)
                ]

            def mm_one(t, kb, half, c):
                nc.tensor.matmul(
                    psums[t][c][:],
                    lhsT=xtiles[t][:, ds(kb * P, P)],
                    rhs=wTs[kb][:, ds(half * HW + c * 512, 512)],
                    start=(kb == 0),
                    stop=(kb == KB - 1),
                )

            def mm(t, kb, half):
                mm_one(t, kb, half, 0)
                mm_one(t, kb, half, 1)

            def close_tile(t, half):
                # both evictions on DVE: start=True matmuls wait on coarsened
                # DVE op-COUNT semaphores, so the eviction engine's stream
                # must contain nothing slower-paced than evictions
                outsb = outsb_pool.tile([P, HW], f32, tag="osb", name=f"osb_{t}_{half}")
                nc.vector.tensor_copy(outsb[:, ds(0, 512)], psums[t][0][:])
                nc.vector.tensor_copy(outsb[:, ds(512, 512)], psums[t][1][:])
                # pass-A outs ride the sync queue (gpsimd still carries the
                # pass-B W quads then); pass-B outs alternate sync/gpsimd so
                # the final drain isn't queued behind the previous tile's
                # 0.5MB (measured ~2.5us of end-of-run out backlog)
                if half == 0 or t % 2 == 0:
                    nc.sync.dma_start(o_d[ds(t * P, P), ds(half * HW, HW)], outsb[:])
                else:
                    nc.gpsimd.dma_start(
                        o_d[ds(t * P, P), ds(half * HW, HW)], outsb[:]
                    )
                del psums[t]

            # ---- prologue: the critical payload (x0-x5 + pass-A W halves,
            # ~7MB) is spread across all three DMA queues — each queue tops
            # out at ~180 GB/s, so one queue alone would gate the prologue
            # at ~30us. Row 0 is chunked for the earliest first matmul;
            # grouped W DMAs keep trigger serialization (~0.7us each) off
            # the pacing path.
            # Each queue moves only ~100 GB/s when all three are active, so
            # the payload is striped round-robin across sync/scalar/gpsimd
            # in strict consumption order (the DVE wait conditions are
            # coarsened to op-counts, so out-of-order arrival cascades).
            # x0/x1 are split into k-strips so their first k-blocks land in
            # ~1us instead of ~5us.
            # the gpsimd queue delivers its first bytes ~10us later than
            # sync/scalar, so the critical payload rides those two only
            def emit_x_striped(t, hi_trig=None):
                xt = x_pool.tile([P, IN_F], bf16, name=f"x_{t}")
                nc.sync.dma_start(xt[:, ds(0, 1024)], xb_d[ds(t * P, P), ds(0, 1024)])
                (hi_trig or nc.scalar).dma_start(
                    xt[:, ds(1024, 1024)], xb_d[ds(t * P, P), ds(1024, 1024)]
                )
                xtiles[t] = xt

            emit_x_striped(0)
            emit_wa_group(0, 1, nc.sync)
            emit_wa_group(1, 1, nc.scalar)
            # x1's upper strip is the gpsimd queue's FIRST item: each queue
            # serves its first ~1MB at only ~50 GB/s (cold start), and this
            # strip isn't needed until ~25us — parking it on gpsimd keeps
            # the scalar queue's W rows 0.25MB earlier
            emit_x_striped(1, nc.gpsimd)
            for kb in range(2, 12):
                emit_wa_group(kb, 1, nc.sync if kb % 2 == 0 else nc.scalar)
            # rows 12-15 are consumed last (~33-39us): park them on gpsimd
            # behind x1's strip, freeing 1MB of cold sync/scalar queue time
            for kb in range(12, KB):
                emit_wa_group(kb, 1, nc.gpsimd)
            emit_x_tile(2)                      # sync
            emit_x_tile(3, nc.scalar)
            emit_x_tile(4, nc.gpsimd)
            emit_x_tile(5)                      # sync

            # PE/HAM warmup: a few fp32 matmuls off the scales tile (first
            # data to arrive) into a scratch PSUM bank, so the HAM clock
            # gate is already at 8/8 when the real stream starts
            warm = ps_pool.tile([P, 512], f32, tag="ps", name="warm")
            for _ in range(5):
                nc.tensor.matmul(
                    warm[:, ds(0, 256)], lhsT=wscr[:, ds(0, P)],
                    rhs=wscr[:, ds(0, 256)], start=True, stop=True,
                )

            # ---- pass A over out[:, 0:1024]: first four tiles interleaved
            # k-block-by-k-block (8 matmuls = ~1.7us of PE work per arriving
            # W row) so the PE never starves during the W load phase.
            for t in range(4):
                open_group(t)
            for kb in range(KB):
                for t in range(4):
                    mm(t, kb, 0)

            def weave(t):
                # spread the x-tile bulk and pass-B W quads across pass A's
                # eviction stream, paced so each quad is in before its ACT
                # dequant rows come up and each x tile before its matmuls:
                # gpsimd carries x6-x9 + the quads, sync carries x10-x15
                # (ahead of that close's demand-paced out-DMA)
                if t == 0:
                    emit_x_tile(6, nc.gpsimd)
                    emit_x_tile(7, nc.gpsimd)
                    emit_wb_load(0, 4)
                elif t == 1:
                    emit_wb_load(4, 4)
                    emit_x_tile(8, nc.gpsimd)
                elif t == 2:
                    emit_wb_load(8, 4)
                    emit_x_tile(9, nc.gpsimd)
                    emit_x_tile(10)
                    emit_x_tile(11)
                elif t == 3:
                    emit_wb_load(12, 4)
                    emit_x_tile(12)
                    emit_x_tile(13)
                elif t == 4:
                    emit_x_tile(14)
                    emit_x_tile(15)
                if 2 * t + 1 < KB:
                    dequant_b_row(2 * t)
                    dequant_b_row(2 * t + 1)

            for t in range(4):
                weave(t)
                close_tile(t, 0)

            for t in range(4, TB):
                open_group(t)
                for kb in range(KB):
                    mm(t, kb, 0)
                weave(t)
                close_tile(t, 0)

            # ---- pass B over out[:, 1024:2048]: pure matmul stream ----
            for t in range(TB):
                open_group(t)
                last = t == TB - 1
                if not last:
                    for kb in range(KB):
                        mm(t, kb, 1)
                    close_tile(t, 1)
                else:
                    # chunk-outer on the final tile so the drain pipelines
                    outsb = outsb_pool.tile([P, HW], f32, tag="osb", name="osb_last")
                    for c in range(2):
                        for kb in range(KB):
                            mm_one(t, kb, 1, c)
                        if c == 0:
                            nc.vector.tensor_copy(
                                outsb[:, ds(0, 512)], psums[t][0][:]
                            )
                            nc.gpsimd.dma_start(
                                o_d[ds(t * P, P), ds(HW, 512)], outsb[:, ds(0, 512)]
                            )
                        else:
                            # final chunk: evict + drain split across queues
                            nc.vector.tensor_copy(
                                outsb[:, ds(512, 256)], psums[t][1][:, ds(0, 256)]
                            )
                            nc.scalar.copy(
                                outsb[:, ds(768, 256)], psums[t][1][:, ds(256, 256)]
                            )
                            nc.sync.dma_start(
                                o_d[ds(t * P, P), ds(HW + 512, 256)],
                                outsb[:, ds(512, 256)],
                            )
                            nc.scalar.dma_start(
                                o_d[ds(t * P, P), ds(HW + 768, 256)],
                                outsb[:, ds(768, 256)],
                            )
                    del psums[t]

    nc.compile()
    return nc


def _get_compiled():
    global _cached
    if _cached is None:
        _cached = _build()
    return _cached


def _ensure_ntff_hook():
    """Register the axon NTFF profile hook (boot skips it when
    antenv.axon_hooks is absent from the image). Only needed for trace=True."""
    import sys as _sys
    import types as _types

    if "antenv.axon_hooks" not in _sys.modules:
        import antenv

        mod = _types.ModuleType("antenv.axon_hooks")
        mod._hook = None

        def set_axon_ntff_profile_hook(h):
            mod._hook = h

        def get_axon_ntff_profile_hook():
            return mod._hook

        mod.set_axon_ntff_profile_hook = set_axon_ntff_profile_hook
        mod.get_axon_ntff_profile_hook = get_axon_ntff_profile_hook
        _sys.modules["antenv.axon_hooks"] = mod
        antenv.axon_hooks = mod
    mod = _sys.modules["antenv.axon_hooks"]
    if mod._hook is None:
        from trn_agent_boot.trn_boot import _ntff_profile_via_ctypes

        hook = _ntff_profile_via_ctypes("/opt/axon/libaxon_pjrt.so")
        if hook is not None:
            mod.set_axon_ntff_profile_hook(hook)


def run(x, weight, weight_scale, trace=False, trace_cores=None):
    from concourse.bass_utils import run_bass_kernel_spmd

    nc = _get_compiled()

    import ml_dtypes

    bf16 = ml_dtypes.bfloat16
    x = np.asarray(x, dtype=np.float32)
    weight = np.asarray(weight, dtype=np.float32)
    wt = np.ascontiguousarray(weight.T.astype(bf16))
    weight_scale = np.asarray(weight_scale, dtype=np.float32)
    # [P, KB(bi), OBL(bo)]: s[p, bi, bo] = weight_scale[bo, bi]
    scales_b = np.ascontiguousarray(
        np.broadcast_to(weight_scale.T[None, :, :], (P, KB, OBL)).astype(np.float32)
    )

    def blocked_x(shard):
        # xb[t, p, kb, m] = shard[t*128+m, kb*128+p]  (layout prep only)
        xb = shard.reshape(TB, P, KB, P).transpose(0, 3, 2, 1)
        return np.ascontiguousarray(xb.astype(bf16).reshape(TB * P, IN_F))

    in_maps = [
        {
            "xb": blocked_x(x[c * TSH : (c + 1) * TSH]),
            "wt": wt,
            "s": scales_b,
        }
        for c in range(NCORES)
    ]
    kwargs = {}
    if trace:
        try:
            _ensure_ntff_hook()
        except Exception as e:  # tracing is best-effort; the run still works
            print(f"ntff hook registration failed ({e}); tracing may be skipped")
        kwargs = dict(trace=True, trace_cores=trace_cores or [0])
    res = run_bass_kernel_spmd(nc, in_maps, core_ids=list(range(NCORES)), **kwargs)
    out = np.concatenate([res.results[c]["out"] for c in range(NCORES)], axis=0)
    return out, res


def kernel(x, weight, weight_scale):
    # Rare transient device errors (NRT_EXEC_UNIT_UNRECOVERABLE) have been
    # observed under the profiling path; retry once to be safe.
    try:
        out, _ = run(x, weight, weight_scale)
    except Exception:
        import time

        time.sleep(2)
        out, _ = run(x, weight, weight_scale)
    return out


# revision 45
# speedup vs baseline: 1.0177x; 1.0035x over previous
"""Trainium2 Bass kernel for MockFP8Linear: out = x @ (W * block_scale)^T.

Strategy: data-parallel over tokens across 8 NeuronCores (no collectives).

Layout: the PE contracts along the partition dim, so both operands need
in_features on partitions. Both are fed to the device pre-transposed as
host-side layout prep (np.ascontiguousarray(.T) + bf16 cast, exactly the
prep class the baseline already used for W):
  - weight: [in, out] bf16. Dequant (per-128x128-block scale) happens
    on-device in one DVE tensor_tensor multiply per k-row, using a
    stride-0 broadcast AP for the scales. W^T (bf16, 8 MB) stays
    resident in SBUF.
  - x: tile-blocked transposed bf16 per-core shard, xb[t, p, kb, m] =
    x[t*128+m, kb*128+p], so each token tile is ONE [128, 4KB-run] DMA
    (DMA engines are packet-rate bound: 256B-run block DMAs measured
    ~6 GB/s/engine vs ~24 GB/s at 4KB runs) and lhsT blocks slice
    straight out of SBUF. No on-device transpose or cast: the
    TensorEngine runs a pure matmul stream.

Main compute runs as TWO PASSES over output halves so the prologue only
gates on half the W dequant: pass A computes out[:, 0:1024] for all 16
token tiles (the first four tiles interleaved k-block-by-k-block so the
PE chases the W-half-row DMA arrivals), pass B computes out[:, 1024:]
as a pure matmul stream over the fully resident operands. Per (tile,
k-block): lhsT(=x^T block, stationary) @ rhs(=W^T slice, moving, N=512)
bf16 matmuls accumulate fp32 into 2 PSUM banks per tile-half (4 tile
groups in flight).

Engine/queue discipline (each lesson trace-measured):
  - The critical prologue payload (x0/x1 split in half-strips + the 16
    pass-A W half-rows, in strict consumption order) rides the sync and
    scalar DMA queues only: per-queue throughput is ~100-180 GB/s
    regardless of engine idle time, and the gpsimd queue delivers its
    first bytes ~10us later than the other two.
  - Pass-A dequant runs on DVE (1.2us per half-row; GPSIMD tensor_tensor
    measured 2.5x-slowing CONCURRENT DVE ops, so it gets none).
  - start=True matmuls wait on coarsened per-engine op-COUNT semaphores,
    so DVE carries evictions + prologue dequants ONLY, in consumption
    order; anything slower-paced on that engine falsely gates PSUM
    bank reuse.
  - Pass-B dequant therefore runs on the otherwise-idle ACT engine as
    per-128-block muls with a [P,1] per-partition scale AP (ACT's scale
    operand can't vary along free dims), woven 2 rows per pass-A close.
  - Pass-B W quads + the x-tile bulk are woven across pass A on the
    gpsimd/sync queues, paced to land just ahead of their consumers;
    all out-DMAs ride sync (demand-paced) to avoid head-of-line
    blocking behind the quads.
  - A short burst of fp32 warmup matmuls off a memset scratch tile
    (no DMA dependency, so it runs ~7-11us and finishes before the first
    real matmul's operands land) pre-warms the HAM clock gate (PE
    defaults to 1.2 GHz until ~3.4us of sustained activity); the last
    tile-half is chunk-pipelined with its drain split across queues to
    shrink the tail. W rows 12-15 (consumed last) ride the slow-starting
    gpsimd queue, freeing 1MB of cold sync/scalar queue time.
"""

import os
import sys

import numpy as np

for _p in ("/opt/trn_rl_repo", "/root/.axon_site/_ro/trn_rl_repo"):
    if os.path.isdir(_p) and _p not in sys.path:
        sys.path.append(_p)

TOKENS, IN_F, OUT_F = 16384, 2048, 2048
NCORES = 8
TSH = TOKENS // NCORES  # tokens per core
P = 128
KB = IN_F // P  # contraction blocks
TB = TSH // P  # token tiles per core
OBL = OUT_F // P  # out_features blocks (scale granularity)
NCH = OUT_F // 512  # psum chunks of the output row-tile

_cached = None


def _build():
    from contextlib import ExitStack

    import concourse.tile as tile
    from concourse import bacc, mybir
    from concourse.bass import ds

    f32 = mybir.dt.float32
    bf16 = mybir.dt.bfloat16

    nc = bacc.Bacc("TRN2", target_bir_lowering=False, debug=False, num_devices=NCORES)
    xb_d = nc.dram_tensor("xb", [TB * P, IN_F], bf16, kind="ExternalInput").ap()
    wt_d = nc.dram_tensor("wt", [IN_F, OUT_F], bf16, kind="ExternalInput").ap()
    s_d = nc.dram_tensor("s", [P, KB, OBL], f32, kind="ExternalInput").ap()
    o_d = nc.dram_tensor("out", [TSH, OUT_F], f32, kind="ExternalOutput").ap()

    with tile.TileContext(nc) as tc:
        with ExitStack() as ctx:
            const = ctx.enter_context(tc.tile_pool(name="const", bufs=1))
            scales = const.tile([P, KB, OBL], f32)
            # only rows 0-3's scale slices are needed in the first ~20us:
            # the other 98KB would push W1/W3 back on the cold scalar queue
            # (its first MB moves at ~30-50 GB/s), so they lead the gpsimd
            # queue instead
            nc.scalar.dma_start(scales[:, ds(0, 4)], s_d[:, ds(0, 4)])
            # warmup scratch: memset needs no DMA, so the HAM warmup matmuls
            # can start at ~7us (right after the DVE preamble) and finish
            # BEFORE the first real matmul's operands land (~11.3us) —
            # sourcing them from the scales tile measured the warmup
            # BLOCKING ~3.8us of ready real work
            wscr = const.tile([P, 512], f32)
            nc.vector.memset(wscr[:], 0.0)

            wT_pool = ctx.enter_context(tc.tile_pool(name="wT", bufs=1))
            wTs = [wT_pool.tile([P, OUT_F], bf16, name=f"wT_{ib}") for ib in range(KB)]

            wnat_pool = ctx.enter_context(tc.tile_pool(name="wnat", bufs=6))
            x_pool = ctx.enter_context(tc.tile_pool(name="x", bufs=1))
            outsb_pool = ctx.enter_context(tc.tile_pool(name="outsb", bufs=3))
            ps_pool = ctx.enter_context(tc.tile_pool(name="ps", bufs=8, space="PSUM"))

            HW = OUT_F // 2  # output columns per pass

            def dequant(kb, src, lo, w):
                # wTs[kb][:, lo:lo+w] = src[:, 0:w] * scale; src 2D contiguous
                nb = w // P
                nc.vector.tensor_tensor(
                    out=wTs[kb][:, ds(lo, w)].rearrange("p (b c) -> p b c", c=P),
                    in0=src[:, ds(0, w)].rearrange("p (b c) -> p b c", c=P),
                    in1=scales[:, kb, ds(lo // P, nb), None].broadcast_to([P, nb, P]),
                    op=mybir.AluOpType.mult,
                )

            def emit_wa_group(kb0, nrows, trig, chunks=1):
                # one trigger (~0.7us of engine time) loads nrows rows' pass-A
                # halves; each row stays a 2D contiguous [P, HW] slice
                wg = wnat_pool.tile(
                    [P, nrows * HW], bf16, tag="wnat", name=f"wa_{kb0}"
                )
                if nrows == 1:
                    cw = HW // chunks
                    for j in range(chunks):
                        trig.dma_start(
                            wg[:, ds(j * cw, cw)],
                            wt_d[ds(kb0 * P, P), ds(j * cw, cw)],
                        )
                        dequant(kb0, wg[:, ds(j * cw, cw)], j * cw, cw)
                    return
                trig.dma_start(
                    wg[:].rearrange("p (r o) -> p r o", o=HW),
                    wt_d[ds(kb0 * P, nrows * P), ds(0, HW)].rearrange(
                        "(r p) o -> p r o", p=P
                    ),
                )
                for r in range(nrows):
                    dequant(kb0 + r, wg[:, ds(r * HW, HW)], 0, HW)

            wbs = {}

            def emit_wb_load(kb0, nrows):
                # pass-B halves: queued on gpsimd right AFTER the critical
                # pass-A payload (ordered queues => no early contention)
                wg = wnat_pool.tile(
                    [P, nrows * HW], bf16, tag="wnat", name=f"wb_{kb0}"
                )
                nc.gpsimd.dma_start(
                    wg[:].rearrange("p (r o) -> p r o", o=HW),
                    wt_d[ds(kb0 * P, nrows * P), ds(HW, HW)].rearrange(
                        "(r p) o -> p r o", p=P
                    ),
                )
                for r in range(nrows):
                    wbs[kb0 + r] = wg[:, ds(r * HW, HW)]

            def dequant_b_row(kb):
                # pass-B dequant on ACT as per-128-block muls with a [P,1]
                # per-partition scale AP (~0.5us each, ACT is otherwise
                # idle, and DVE must stay evictions-only)
                src_row = wbs[kb]
                for bo in range(OBL // 2):
                    nc.scalar.mul(
                        wTs[kb][:, ds(HW + bo * P, P)],
                        src_row[:, ds(bo * P, P)],
                        scales[:, kb, HW // P + bo, None],
                    )

            xtiles = {}

            def emit_x_tile(t, trig=None):
                xt = x_pool.tile([P, IN_F], bf16, name=f"x_{t}")
                (trig or nc.sync).dma_start(xt[:], xb_d[ds(t * P, P), :])
                xtiles[t] = xt

            psums = {}

            def open_group(t):
                psums[t] = [
                    ps_pool.tile([P, 512], f32, tag="ps", name=f"ps_{t}_{c}")
                    for c in range(2)
                ]

            def mm_one(t, kb, half, c):
                nc.tensor.matmul(
                    psums[t][c][:],
                    lhsT=xtiles[t][:, ds(kb * P, P)],
                    rhs=wTs[kb][:, ds(half * HW + c * 512, 512)],
                    start=(kb == 0),
                    stop=(kb == KB - 1),
                )

            def mm(t, kb, half):
                mm_one(t, kb, half, 0)
                mm_one(t, kb, half, 1)

            def close_tile(t, half):
                # both evictions on DVE: start=True matmuls wait on coarsened
                # DVE op-COUNT semaphores, so the eviction engine's stream
                # must contain nothing slower-paced than evictions
                outsb = outsb_pool.tile([P, HW], f32, tag="osb", name=f"osb_{t}_{half}")
                nc.vector.tensor_copy(outsb[:, ds(0, 512)], psums[t][0][:])
                nc.vector.tensor_copy(outsb[:, ds(512, 512)], psums[t][1][:])
                # pass-A outs ride the sync queue (gpsimd still carries the
                # pass-B W quads then); pass-B outs alternate sync/gpsimd so
                # the final drain isn't queued behind the previous tile's
                # 0.5MB (measured ~2.5us of end-of-run out backlog)
                if half == 0 or t % 2 == 0:
                    nc.sync.dma_start(o_d[ds(t * P, P), ds(half * HW, HW)], outsb[:])
                else:
                    nc.gpsimd.dma_start(
                        o_d[ds(t * P, P), ds(half * HW, HW)], outsb[:]
                    )
                del psums[t]

            # ---- prologue: the critical payload (x0-x5 + pass-A W halves,
            # ~7MB) is spread across all three DMA queues — each queue tops
            # out at ~180 GB/s, so one queue alone would gate the prologue
            # at ~30us. Row 0 is chunked for the earliest first matmul;
            # grouped W DMAs keep trigger serialization (~0.7us each) off
            # the pacing path.
            # Each queue moves only ~100 GB/s when all three are active, so
            # the payload is striped round-robin across sync/scalar/gpsimd
            # in strict consumption order (the DVE wait conditions are
            # coarsened to op-counts, so out-of-order arrival cascades).
            # x0/x1 are split into k-strips so their first k-blocks land in
            # ~1us instead of ~5us.
            # the gpsimd queue delivers its first bytes ~10us later than
            # sync/scalar, so the critical payload rides those two only
            def emit_x_striped(t, hi_trig=None):
                xt = x_pool.tile([P, IN_F], bf16, name=f"x_{t}")
                nc.sync.dma_start(xt[:, ds(0, 1024)], xb_d[ds(t * P, P), ds(0, 1024)])
                (hi_trig or nc.scalar).dma_start(
                    xt[:, ds(1024, 1024)], xb_d[ds(t * P, P), ds(1024, 1024)]
                )
                xtiles[t] = xt

            nc.gpsimd.dma_start(scales[:, ds(4, KB - 4)], s_d[:, ds(4, KB - 4)])
            emit_x_striped(0)
            emit_wa_group(0, 1, nc.sync)
            emit_wa_group(1, 1, nc.scalar)
            # x1's upper strip is the gpsimd queue's FIRST item: each queue
            # serves its first ~1MB at only ~50 GB/s (cold start), and this
            # strip isn't needed until ~25us — parking it on gpsimd keeps
            # the scalar queue's W rows 0.25MB earlier
            emit_x_striped(1, nc.gpsimd)
            for kb in range(2, 12):
                emit_wa_group(kb, 1, nc.sync if kb % 2 == 0 else nc.scalar)
            # rows 12-15 are consumed last (~33-39us): park them on gpsimd
            # behind x1's strip, freeing 1MB of cold sync/scalar queue time
            for kb in range(12, KB):
                emit_wa_group(kb, 1, nc.gpsimd)
            emit_x_tile(2)                      # sync
            emit_x_tile(3, nc.scalar)
            emit_x_tile(4, nc.gpsimd)
            emit_x_tile(5)                      # sync

            # PE/HAM warmup: a few fp32 matmuls off the scales tile (first
            # data to arrive) into a scratch PSUM bank, so the HAM clock
            # gate is already at 8/8 when the real stream starts
            warm = ps_pool.tile([P, 512], f32, tag="ps", name="warm")
            for _ in range(5):
                nc.tensor.matmul(
                    warm[:, ds(0, 256)], lhsT=wscr[:, ds(0, P)],
                    rhs=wscr[:, ds(0, 256)], start=True, stop=True,
                )

            # ---- pass A over out[:, 0:1024]: first four tiles interleaved
            # k-block-by-k-block (8 matmuls = ~1.7us of PE work per arriving
            # W row) so the PE never starves during the W load phase.
            for t in range(4):
                open_group(t)
            for kb in range(KB):
                for t in range(4):
                    mm(t, kb, 0)

            def weave(t):
                # spread the x-tile bulk and pass-B W quads across pass A's
                # eviction stream, paced so each quad is in before its ACT
                # dequant rows come up and each x tile before its matmuls:
                # gpsimd carries x6-x9 + the quads, sync carries x10-x15
                # (ahead of that close's demand-paced out-DMA)
                if t == 0:
                    emit_x_tile(6, nc.gpsimd)
                    emit_x_tile(7, nc.gpsimd)
                    emit_wb_load(0, 4)
                elif t == 1:
                    emit_wb_load(4, 4)
                    emit_x_tile(8, nc.gpsimd)
                elif t == 2:
                    emit_wb_load(8, 4)
                    emit_x_tile(9, nc.gpsimd)
                    emit_x_tile(10)
                    emit_x_tile(11)
                elif t == 3:
                    emit_wb_load(12, 4)
                    emit_x_tile(12)
                    emit_x_tile(13)
                elif t == 4:
                    emit_x_tile(14)
                    emit_x_tile(15)
                if 2 * t + 1 < KB:
                    dequant_b_row(2 * t)
                    dequant_b_row(2 * t + 1)

            for t in range(4):
                weave(t)
                close_tile(t, 0)

            for t in range(4, TB):
                open_group(t)
                for kb in range(KB):
                    mm(t, kb, 0)
                weave(t)
                close_tile(t, 0)

            # ---- pass B over out[:, 1024:2048]: pure matmul stream ----
            for t in range(TB):
                open_group(t)
                last = t == TB - 1
                if not last:
                    for kb in range(KB):
                        mm(t, kb, 1)
                    close_tile(t, 1)
                else:
                    # chunk-outer on the final tile so the drain pipelines
                    outsb = outsb_pool.tile([P, HW], f32, tag="osb", name="osb_last")
                    for c in range(2):
                        for kb in range(KB):
                            mm_one(t, kb, 1, c)
                        if c == 0:
                            nc.vector.tensor_copy(
                                outsb[:, ds(0, 512)], psums[t][0][:]
                            )
                            nc.gpsimd.dma_start(
                                o_d[ds(t * P, P), ds(HW, 512)], outsb[:, ds(0, 512)]
                            )
                        else:
                            # final chunk: evict + drain split across queues
                            nc.vector.tensor_copy(
                                outsb[:, ds(512, 256)], psums[t][1][:, ds(0, 256)]
                            )
                            nc.scalar.copy(
                                outsb[:, ds(768, 256)], psums[t][1][:, ds(256, 256)]
                            )
                            nc.sync.dma_start(
                                o_d[ds(t * P, P), ds(HW + 512, 256)],
                                outsb[:, ds(512, 256)],
                            )
                            nc.scalar.dma_start(
                                o_d[ds(t * P, P), ds(HW + 768, 256)],
                                outsb[:, ds(768, 256)],
                            )
                    del psums[t]

    nc.compile()
    return nc


def _get_compiled():
    global _cached
    if _cached is None:
        _cached = _build()
    return _cached


def _ensure_ntff_hook():
    """Register the axon NTFF profile hook (boot skips it when
    antenv.axon_hooks is absent from the image). Only needed for trace=True."""
    import sys as _sys
    import types as _types

    if "antenv.axon_hooks" not in _sys.modules:
        import antenv

        mod = _types.ModuleType("antenv.axon_hooks")
        mod._hook = None

        def set_axon_ntff_profile_hook(h):
            mod._hook = h

        def get_axon_ntff_profile_hook():
            return mod._hook

        mod.set_axon_ntff_profile_hook = set_axon_ntff_profile_hook
        mod.get_axon_ntff_profile_hook = get_axon_ntff_profile_hook
        _sys.modules["antenv.axon_hooks"] = mod
        antenv.axon_hooks = mod
    mod = _sys.modules["antenv.axon_hooks"]
    if mod._hook is None:
        from trn_agent_boot.trn_boot import _ntff_profile_via_ctypes

        hook = _ntff_profile_via_ctypes("/opt/axon/libaxon_pjrt.so")
        if hook is not None:
            mod.set_axon_ntff_profile_hook(hook)


def run(x, weight, weight_scale, trace=False, trace_cores=None):
    from concourse.bass_utils import run_bass_kernel_spmd

    nc = _get_compiled()

    import ml_dtypes

    bf16 = ml_dtypes.bfloat16
    x = np.asarray(x, dtype=np.float32)
    weight = np.asarray(weight, dtype=np.float32)
    wt = np.ascontiguousarray(weight.T.astype(bf16))
    weight_scale = np.asarray(weight_scale, dtype=np.float32)
    # [P, KB(bi), OBL(bo)]: s[p, bi, bo] = weight_scale[bo, bi]
    scales_b = np.ascontiguousarray(
        np.broadcast_to(weight_scale.T[None, :, :], (P, KB, OBL)).astype(np.float32)
    )

    def blocked_x(shard):
        # xb[t, p, kb, m] = shard[t*128+m, kb*128+p]  (layout prep only)
        xb = shard.reshape(TB, P, KB, P).transpose(0, 3, 2, 1)
        return np.ascontiguousarray(xb.astype(bf16).reshape(TB * P, IN_F))

    in_maps = [
        {
            "xb": blocked_x(x[c * TSH : (c + 1) * TSH]),
            "wt": wt,
            "s": scales_b,
        }
        for c in range(NCORES)
    ]
    kwargs = {}
    if trace:
        try:
            _ensure_ntff_hook()
        except Exception as e:  # tracing is best-effort; the run still works
            print(f"ntff hook registration failed ({e}); tracing may be skipped")
        kwargs = dict(trace=True, trace_cores=trace_cores or [0])
    res = run_bass_kernel_spmd(nc, in_maps, core_ids=list(range(NCORES)), **kwargs)
    out = np.concatenate([res.results[c]["out"] for c in range(NCORES)], axis=0)
    return out, res


def kernel(x, weight, weight_scale):
    # Rare transient device errors (NRT_EXEC_UNIT_UNRECOVERABLE) have been
    # observed under the profiling path; retry once to be safe.
    try:
        out, _ = run(x, weight, weight_scale)
    except Exception:
        import time

        time.sleep(2)
        out, _ = run(x, weight, weight_scale)
    return out
